# revision 1
# baseline (speedup 1.0000x reference)
"""Trainium2 Bass kernel for a 3-layer GCN encoder (B=32, N=1000, D=256).

Math: the reference's normalized adjacency for a fully-connected graph
(self_loop=False -> adj = ones) is A_norm = ones(N,N)/N, so the
"aggregation" einsum is a mean over nodes broadcast back to every node.
Since mean o linear = linear o mean and the mean is idempotent across
layers (h is constant over nodes after layer 0), the whole network
collapses to, per batch b:

    m_b  = mean_n node_feature[b, n, :]          # (D,)
    h1_b = relu(m_b @ W0 + b0)
    h2_b = relu(h1_b @ W1 + b1)
    h3_b = h2_b @ W2 + b2
    out[b, n, :] = node_feature[b, n, :] + h3_b  # broadcast residual

Sharding: data-parallel over batch, 4 batches per core on 8 cores.

v6 dataflow (per core), all knobs HW-A/B'd on the axon trn2 pool:
- Host stages node_feature as f16 (tolerance is 2e-2; f16 staging costs
  ~3e-4), so the device moves 2MB in + 2MB out per core instead of 8MB.
- Node split "(p t)": partition p holds nodes 8p..8p+7 -> every DMA
  descriptor is one contiguous 4KB run, tiling the batch sequentially.
- Loads on the SWDGE (gpsimd) queue (streams back-to-back with no
  per-DMA ring overhead; measured faster than HWDGE), stores on the
  HWDGE rings: each DMA queue carries one direction only.
- Per-pair chain (CHAIN_GROUP=2): PE matmuls (f16, PSUM f32), ACT does
  every PSUM->SBUF copy/bias/relu (per-partition bias fits the column
  orientation), layer 2 in row orientation (no transpose), bias b2
  folded into the rank-1 broadcast matmul. DVE touches nothing but the
  residual adds, so the in-order engine queues never cross-block.
- Residual adds in-place on DVE (all-f16 SBUF operands -> fast mode)
  with 2 of 8 slices per batch on gpsimd.
- Output is f16 on device; host upcasts to float32.
"""

import numpy as np

import concourse.bacc as bacc
import concourse.bass as bass
import concourse.mybir as mybir
import concourse.tile as tile
from concourse.bass_utils import run_bass_kernel_spmd

F32 = mybir.dt.float32
F16 = mybir.dt.float16

B, N, D, L = 32, 1000, 256, 3
NCORES = 8
NB = B // NCORES  # batches per core
P = 125           # partition rows per node-slice
T = N // P        # node-slices per partition row
HALF = 128        # half of D

# tuning knobs (HW A/B)
POOL_ADD_SLICES = 2  # residual-add t-slices per batch offloaded to gpsimd
CHAIN_GROUP = 2      # batches whose chains are computed together
LOADS = "swdge"      # "swdge" (loads on gpsimd queue) or "hwdge" (sync/scalar)
STORE_MODE = "mix"   # "2rings" | "1ring" | "mix" (b2/b3 via drained SWDGE)
STORE_HALVES = True  # store each batch as two 256KB halves vs one 512KB DMA
LOAD_HALVES = True   # load each batch as two 256KB DMAs (earlier sums start)
SHAPE_SCHED = False  # force per-group-sequential engine order via wait floors

_NC_CACHE = {}


def _build_nc(reps=1):
    nc = bacc.Bacc("TRN2", target_bir_lowering=False, debug=False)

    nf_d = nc.dram_tensor("nf", [NB, N, D], F16, kind="ExternalInput")
    w_d = nc.dram_tensor("w", [L, D, D], F16, kind="ExternalInput")
    bvec_d = nc.dram_tensor("bvec", [HALF, 2 * L], F32, kind="ExternalInput")
    b2row_d = nc.dram_tensor("b2row", [1, D], F16, kind="ExternalInput")
    out_d = nc.dram_tensor("out", [NB, N, D], F16, kind="ExternalOutput")

    ones_row_d = nc.inline_tensor(np.ones((1, P), np.float16), "ones_row")

    COPY = mybir.ActivationFunctionType.Copy
    RELU = mybir.ActivationFunctionType.Relu

    G = CHAIN_GROUP
    NG = NB // G

    with tile.TileContext(nc) as tc:
        with (
            tc.tile_pool(name="const", bufs=1) as cpool,
            tc.tile_pool(name="data", bufs=NB) as dpool,
            tc.tile_pool(name="vec", bufs=4) as vpool,
            tc.tile_pool(name="ps_sum", bufs=2, space=bass.MemorySpace.PSUM) as ps_sum,
            tc.tile_pool(name="ps_chain", bufs=2, space=bass.MemorySpace.PSUM) as ps_chain,
            tc.tile_pool(name="ps_row", bufs=2, space=bass.MemorySpace.PSUM) as ps_row,
            tc.tile_pool(name="ps_bc", bufs=2, space=bass.MemorySpace.PSUM) as ps_bc,
        ):
            # ---- constants ----
            ones_col = cpool.tile([P, 1], F16, tag="ones_col", name="ones_col")
            nc.vector.memset(ones_col[:], 1.0)
            ones_row = cpool.tile([1, P], F16, tag="ones_row", name="ones_row")
            nc.sync.dma_start(ones_row[:], ones_row_d[:])
            b2row = cpool.tile([1, D], F16, tag="b2row", name="b2row")
            nc.sync.dma_start(b2row[:], b2row_d[:])
            bvec = cpool.tile([HALF, 2 * L], F32, tag="bvec", name="bvec")
            nc.sync.dma_start(bvec[:], bvec_d[:])
            w_sb = []
            for l in range(L):
                wt = cpool.tile([HALF, 2, D], F16, tag=f"w{l}", name=f"w{l}")
                eng = nc.sync if l == 0 else nc.scalar
                eng.dma_start(wt[:], w_d[l].rearrange("(kc k) e -> k kc e", k=HALF))
                w_sb.append(wt)

            def store_engine(b):
                if LOADS != "swdge":
                    return nc.gpsimd
                if STORE_MODE == "2rings":
                    return nc.sync if b % 2 == 0 else nc.scalar
                if STORE_MODE == "mix":
                    return [nc.sync, nc.scalar, nc.gpsimd, nc.gpsimd][b]
                return nc.sync

            def do_store(b, nf_t, half):
                dst = out_d[b].rearrange("(p t) d -> p t d", p=P)
                store_engine(b).dma_start(
                    dst[:, half * 4:half * 4 + 4, :],
                    nf_t[:, half * 4:half * 4 + 4, :],
                )

            def batch_body():
                for g in range(NG):
                    bs_ = range(g * G, (g + 1) * G)
                    tc.tile_set_cur_wait(0, enable=SHAPE_SCHED)
                    nf_ts, ps_s = [], ps_sum.tile(
                        [HALF, 2 * G], F32, tag="ps_s", name=f"ps_s{g}"
                    )
                    for bi, b in enumerate(bs_):
                        nf_t = dpool.tile([P, T, D], F16, tag="nf", name=f"nf{b}")
                        src = nf_d[b].rearrange("(p t) d -> p t d", p=P)
                        if LOADS == "swdge":
                            if LOAD_HALVES:
                                nc.gpsimd.dma_start(
                                    nf_t[:, 0:T // 2, :], src[:, 0:T // 2, :]
                                )
                                nc.gpsimd.dma_start(
                                    nf_t[:, T // 2:T, :], src[:, T // 2:T, :]
                                )
                            else:
                                nc.gpsimd.dma_start(nf_t[:], src)
                        else:
                            (nc.sync if b % 2 == 0 else nc.scalar).dma_start(
                                nf_t[:], src
                            )
                        nf_ts.append(nf_t)

                        tc.tile_set_cur_wait(0.05 * g, enable=SHAPE_SCHED)
                        # column sums of batch b -> column mh*G + bi
                        for mh in range(2):
                            col = mh * G + bi
                            for t in range(T):
                                nc.tensor.matmul(
                                    ps_s[:, col:col + 1],
                                    nf_t[:, t, mh * HALF:(mh + 1) * HALF],
                                    ones_col[:],
                                    start=(t == 0),
                                    stop=(t == T - 1),
                                )
                        tc.tile_set_cur_wait(0, enable=SHAPE_SCHED)

                    tc.tile_set_cur_wait(0.05 * g, enable=SHAPE_SCHED)
                    # chain for the group; columns mh*G+bi
                    hc = vpool.tile([HALF, 2 * G], F16, tag="h", name=f"sum{g}")
                    nc.scalar.activation(hc[:], ps_s[:], COPY)
                    for l in range(L - 1):
                        pcs = []
                        for mh in range(2):
                            pc = ps_chain.tile(
                                [HALF, G], F32, tag="ps_c", name=f"ps_c{g}_{l}_{mh}"
                            )
                            for kc in range(2):
                                nc.tensor.matmul(
                                    pc[:],
                                    w_sb[l][:, kc, mh * HALF:(mh + 1) * HALF],
                                    hc[:, kc * G:(kc + 1) * G],
                                    start=(kc == 0),
                                    stop=(kc == 1),
                                )
                            pcs.append(pc)
                        hn = vpool.tile([HALF, 2 * G], F16, tag="h", name=f"h{g}_{l}")
                        for mh in range(2):
                            nc.scalar.activation(
                                hn[:, mh * G:(mh + 1) * G],
                                pcs[mh][:],
                                RELU,
                                bias=bvec[:, 2 * l + mh:2 * l + mh + 1],
                            )
                        hc = hn

                    # layer 2 per batch in row orientation + rank-1 broadcast
                    for bi, b in enumerate(bs_):
                        pr = ps_row.tile([1, D], F32, tag="ps_r", name=f"ps_r{b}")
                        for kc in range(2):
                            nc.tensor.matmul(
                                pr[:],
                                hc[:, kc * G + bi:kc * G + bi + 1],
                                w_sb[L - 1][:, kc, :],
                                start=(kc == 0),
                                stop=(kc == 1),
                            )
                        h3r = vpool.tile([1, D], F16, tag="h3r", name=f"h3r{b}")
                        nc.scalar.activation(h3r[:], pr[:], COPY)

                        pb = ps_bc.tile([P, D], F32, tag="ps_b", name=f"ps_b{b}")
                        nc.tensor.matmul(
                            pb[:], ones_row[:], h3r[:], start=True, stop=False
                        )
                        nc.tensor.matmul(
                            pb[:], ones_row[:], b2row[:], start=False, stop=True
                        )
                        pb16 = vpool.tile([P, D], F16, tag="pb16", name=f"pb16{b}")
                        nc.scalar.activation(pb16[:], pb[:], COPY)

                        nf_t = nf_ts[bi]
                        for half in range(2):
                            for t in range(half * 4, half * 4 + 4):
                                eng = (
                                    nc.gpsimd
                                    if t % (4 // max(1, POOL_ADD_SLICES // 2)) == 0
                                    and POOL_ADD_SLICES
                                    else nc.vector
                                )
                                eng.tensor_add(
                                    nf_t[:, t, :], nf_t[:, t, :], pb16[:]
                                )
                            if STORE_HALVES:
                                do_store(b, nf_t, half)
                        if not STORE_HALVES:
                            dst = out_d[b].rearrange("(p t) d -> p t d", p=P)
                            store_engine(b).dma_start(dst, nf_t[:])
                    tc.tile_set_cur_wait(0, enable=SHAPE_SCHED)

            if reps == 1:
                batch_body()
            else:
                with tc.For_i(0, reps, 1):
                    batch_body()

    nc.compile()
    return nc


def _get_nc(reps=1):
    if reps not in _NC_CACHE:
        _NC_CACHE[reps] = _build_nc(reps)
    return _NC_CACHE[reps]


def _make_in_maps(node_feature, Ws, bs):
    nf16 = np.ascontiguousarray(
        np.asarray(node_feature, dtype=np.float32).astype(np.float16)
    )
    w = np.asarray(Ws, dtype=np.float32).copy()
    w[0] *= 1.0 / N  # fold the mean's 1/N into the first layer's weights
    w16 = np.ascontiguousarray(w.astype(np.float16))
    b = np.asarray(bs, dtype=np.float32)
    # bvec[p, 2*l + half] = bs[l, half*128 + p]
    bvec = np.ascontiguousarray(
        b.reshape(L, 2, HALF).transpose(2, 0, 1).reshape(HALF, 2 * L)
    )
    b2row = np.ascontiguousarray(b[L - 1].reshape(1, D).astype(np.float16))
    in_maps = []
    for i in range(NCORES):
        in_maps.append(
            {
                "nf": np.ascontiguousarray(nf16[i * NB:(i + 1) * NB]),
                "w": w16,
                "bvec": bvec,
                "b2row": b2row,
            }
        )
    return in_maps


def run_on_hw(node_feature, Ws, bs):
    import os

    # The NTFF trace hook (antenv.axon_hooks) does not exist in this
    # container; make sure an inherited BASS_TRACE can't pull it in.
    os.environ["BASS_NEVER_TRACE"] = "1"
    nc = _get_nc()
    res = run_bass_kernel_spmd(
        nc,
        _make_in_maps(node_feature, Ws, bs),
        list(range(NCORES)),
        trace=False,
    )
    out = np.concatenate(
        [np.asarray(res.results[i]["out"]) for i in range(NCORES)], axis=0
    ).astype(np.float32)
    return out, res


def kernel(x, node_feature, Ws, bs):
    node_feature = np.asarray(node_feature, dtype=np.float32)
    out, _ = run_on_hw(node_feature, Ws, bs)
    return out, node_feature


# ---------------------------------------------------------------------------
# Timing runner: same PJRT path as run_bass_kernel_spmd under axon, but with
# the jitted executable cached so repeated executions can be timed without
# re-tracing/re-compiling. Used by test.py only.
# ---------------------------------------------------------------------------


class _Runner:
    def __init__(self, nc=None):
        import jax
        from jax.experimental.shard_map import shard_map
        from jax.sharding import Mesh, NamedSharding, PartitionSpec

        from concourse.bass2jax import (
            _bass_exec_p,
            install_neuronx_cc_hook,
            partition_id_tensor,
        )

        install_neuronx_cc_hook()
        self.jax = jax
        if nc is None:
            nc = _get_nc(1)
        partition_name = (
            nc.partition_id_tensor.name if nc.partition_id_tensor else None
        )
        in_names, out_names, out_avals, zero_outs = [], [], [], []
        for alloc in nc.m.functions[0].allocations:
            if not isinstance(alloc, mybir.MemoryLocationSet):
                continue
            name = alloc.memorylocations[0].name
            if alloc.kind == "ExternalInput":
                if name != partition_name:
                    in_names.append(name)
            elif alloc.kind == "ExternalOutput":
                shape = tuple(alloc.tensor_shape)
                dt = mybir.dt.np(alloc.dtype)
                out_names.append(name)
                out_avals.append(jax.core.ShapedArray(shape, dt))
                zero_outs.append(np.zeros(shape, dt))
        self.in_names = in_names
        self.out_names = out_names
        self.out_avals = out_avals
        self.zero_outs = zero_outs
        n_params, n_outs = len(in_names), len(out_names)
        all_names = tuple(
            in_names + out_names + ([partition_name] if partition_name else [])
        )

        def _body(*args):
            operands = list(args)
            if partition_name is not None:
                operands.append(partition_id_tensor())
            outs = _bass_exec_p.bind(
                *operands,
                out_avals=tuple(out_avals),
                in_names=all_names,
                out_names=tuple(out_names),
                lowering_input_output_aliases=(),
                sim_require_finite=True,
                sim_require_nnan=True,
                nc=nc,
            )
            return tuple(outs)

        devices = jax.devices()[:NCORES]
        self.mesh = Mesh(np.asarray(devices), ("core",))
        self.sharding = NamedSharding(self.mesh, PartitionSpec("core"))
        in_specs = (PartitionSpec("core"),) * (n_params + n_outs)
        out_specs = (PartitionSpec("core"),) * n_outs
        self.jitted = jax.jit(
            shard_map(
                _body,
                mesh=self.mesh,
                in_specs=in_specs,
                out_specs=out_specs,
                check_rep=False,
            ),
            donate_argnums=tuple(range(n_params, n_params + n_outs)),
            keep_unused=True,
        )

    def stage_inputs(self, in_maps):
        concat = [
            np.concatenate([m[name] for m in in_maps], axis=0)
            for name in self.in_names
        ]
        return [self.jax.device_put(a, self.sharding) for a in concat]

    def stage_zeros(self):
        return [
            self.jax.device_put(
                np.zeros((NCORES * z.shape[0], *z.shape[1:]), z.dtype), self.sharding
            )
            for z in self.zero_outs
        ]

    def run(self, dev_inputs, dev_zeros):
        return self.jitted(*dev_inputs, *dev_zeros)


_RUNNER_CACHE = {}


def get_runner(reps=1):
    if reps not in _RUNNER_CACHE:
        _RUNNER_CACHE[reps] = _Runner(_get_nc(reps))
    return _RUNNER_CACHE[reps]

